# revision 25
# baseline (speedup 1.0000x reference)
"""LlamaAttention (B=2, S=2048, H=4096, NH=32) on 8 Trainium2 NeuronCores.

Sharding: tensor-parallel over heads (4 heads / core). Column-parallel
Wq/Wk/Wv, row-parallel Wo; the Wo partial sums are reduced on the host
(the all-reduce of the TP recipe, done during unshard).

All bulk tensors move as bf16 (matmul is full-rate for bf16, so this
halves HBM traffic at no compute cost); accumulation stays fp32 in PSUM.

Per-core dataflow:
  phase 1: Wq/Wk/Wv resident in SBUF (bf16); X^T streamed once in
           [4096, 512]-token chunks. Q^T,K^T = RoPE(W^T-chunk @ X^T)
           -> DRAM [d, t]; V = X^T-chunk^T @ WvT -> DRAM [t, d].
  phase 2: per head: S^T[k,q] = K^T-tile^T @ Q^T (contraction d=128).
           Only non-fully-masked 128(k)x512(q) blocks are computed, and
           diagonal blocks are trimmed to their valid q-subrange. The 16
           distinct diagonal mask tiles are cached in SBUF. exp on ACT;
           denominators via ones-matmul; ctx^T[d,q] = V-tile^T @ expS^T.
  phase 3: O^T partial = WoT-tile^T @ ctx^T -> DRAM [o, t] bf16, with the
           Wo tile loop outermost so Wo is loaded exactly once.

Host side: pre-transposes X and the weights (layout marshaling, bf16
cast), builds the block structure from the attention mask, sums the 8
partial O^T outputs in fp32 and transposes back.
"""
import sys

sys.path.insert(0, "/opt/trn_rl_repo")

import numpy as np
import ml_dtypes

import concourse.bass as bass
import concourse.bacc as bacc
import concourse.tile as tile
import concourse.mybir as mybir

B, S, H, NH = 2, 2048, 4096, 32
HD = H // NH          # 128
NC = 8                # cores
DL = H // NC          # 512 local dims (4 heads / core)
NHL = NH // NC        # 4 local heads
BT = B * S            # 4096 tokens
P = 128
CH = 512              # phase-1 X^T chunk (matmul moving dim)
QT = 512              # phase-2 query tile (free dim)
KT = 128              # phase-2 key tile (partition dim)
NKO = H // P          # 32 contraction subtiles

DT = mybir.dt.float32
BF = mybir.dt.bfloat16
F32 = mybir.dt.float32
AF = mybir.ActivationFunctionType
NPBF = ml_dtypes.bfloat16


def _phase1(nc, tc, pools, aps, scratches, mask_load):
    """QKV projections + RoPE in [token, dim] orientation: the X-chunk tile
    is the stationary operand shared by the Q, K and V matmuls of each
    (tsub, hs) step — one PE weight load feeds three matmuls (the sim
    charges Ldweights as free, hardware does not). RoPE rotate-half acts on
    free-dim halves (sin tables carry the rotation sign). Q/K are then
    PE-transposed back to [d, t] before the scratch store, so phase 2 keeps
    its fast contiguous loads (XBAR transpose-DMA loads are slow on HW)."""
    p1x, p1w, p1t, p1r, psA, psV, psT = pools
    xt3, wq3, wk3, wv3, tabs_ap, ident = aps

    def _load_w(w3, nm):
        lst = []
        for q4 in range(4):
            t = p1w.tile([P, NKO // 4, DL], BF, tag=f"{nm}{q4}",
                         name=f"{nm}{q4}")
            nc.sync.dma_start(t[:], w3[:, bass.ds(q4 * (NKO // 4), NKO // 4), :])
            lst.append(t)
        return lst

    xcs, tts = {}, {}

    def _load_xc(c):
        xc = p1x.tile([P, NKO, CH], BF, tag="xt", name=f"xt{c}")
        nc.sync.dma_start(xc[:], xt3[:, :, bass.ds(c * CH, CH)])
        xcs[c] = xc

    def _load_tabs(c):
        # per-chunk table tiles [t, d]: [128, CH//P tsub, NHL, 2, 64]
        tloc = (c % (S // CH)) * CH
        tt = {}
        for nm, t_ap in tabs_ap.items():
            t = p1t.tile([P, CH // P, NHL, 2, 64], BF, tag="tab_" + nm,
                         name=f"tab_{nm}{c}")
            nc.sync.dma_start(
                t[:], t_ap[bass.ds(tloc, CH), :].rearrange(
                    "(ts p) (h two s) -> p ts h two s", p=P, h=NHL, two=2))
            tt[nm] = t
        tts[c] = tt

    wq_r = _load_w(wq3, "wq")
    _load_xc(0)
    _load_tabs(0)
    wk_r = _load_w(wk3, "wk")
    wv_r = _load_w(wv3, "wv")
    mask_load()  # phase-2 mask tiles: queue behind phase-1-critical loads

    w_res = {"q": wq_r, "k": wk_r, "v": wv_r}
    for c in range(BT // CH):                     # 8 chunks of 512 tokens
        b, tloc = c // (S // CH), (c % (S // CH)) * CH
        qt_d, kt_d, v_d = scratches[b]
        if c + 1 < BT // CH:
            _load_xc(c + 1)
            _load_tabs(c + 1)
        xc = xcs.pop(c)
        tabs = tts.pop(c)
        for tsub in range(CH // P):               # 4 token subtiles
            ps = {}
            ps["q"] = psA.tile([P, DL], F32, tag="psq", name="psq")
            ps["k"] = psA.tile([P, DL], F32, tag="psk", name="psk")
            ps["v"] = psV.tile([P, DL], F32, tag="psv", name="psv")
            for hs in range(NKO):
                nc.tensor.ldweights(xc[:, hs, bass.ts(tsub, P)])
                for nm in ("q", "k", "v"):
                    mm = nc.tensor.matmul(
                        ps[nm][:], xc[:, hs, bass.ts(tsub, P)],
                        w_res[nm][hs // 8][:, hs % 8, :],
                        start=(hs == 0), stop=(hs == NKO - 1))
                    mm.ins.ldweights = False
            trow = bass.ds(tloc + tsub * P, P)
            for nm, outd in (("q", qt_d), ("k", kt_d)):
                pv = ps[nm][:].rearrange("p (h two s) -> p h two s",
                                         h=NHL, two=2)
                cosT = tabs["c"][:, tsub]
                sinT = tabs["s"][:, tsub]
                rc = p1r.tile([P, NHL, 2, 64], F32, tag="rc")
                rs = p1r.tile([P, NHL, 2, 64], F32, tag="rs")
                nc.vector.tensor_mul(rc[:], pv, cosT)
                # rotate-half via swapped half read; sign folded into sinT
                nc.vector.tensor_mul(
                    rs[:, :, 0, :], pv[:, :, 1, :], sinT[:, :, 0, :])
                nc.vector.tensor_mul(
                    rs[:, :, 1, :], pv[:, :, 0, :], sinT[:, :, 1, :])
                ro = p1r.tile([P, DL], BF, tag="ro")
                nc.vector.tensor_tensor(
                    ro[:].rearrange("p (h two s) -> p h two s", h=NHL, two=2),
                    rc[:], rs[:], mybir.AluOpType.add)
                # 4 PE transposes -> one psum tile -> one copy + one DMA
                pst = psT.tile([P, 4, P], BF, tag="pst", name="pst")
                for dsub in range(DL // P):
                    nc.tensor.transpose(pst[:, dsub, :], ro[:, bass.ts(dsub, P)],
                                        ident[:])
                rt = p1r.tile([P, 4, P], BF, tag="rt")
                nc.scalar.copy(rt[:], pst[:])
                nc.sync.dma_start(
                    outd[:, trow].rearrange("(ds p) t -> p ds t", p=P), rt[:])
            vo = p1r.tile([P, DL], BF, tag="vo")
            nc.scalar.copy(vo[:], ps["v"][:])
            nc.sync.dma_start(v_d[trow, :], vo[:])


def _phase2_batch(nc, tc, b, spec, pools, mask_r, ones_r, scratch, ctxT):
    """Attention for batch b -> ctxT [P, NHL, S] (bf16)."""
    p2, p2e, psS, psSum, psC = pools
    qt_d, kt_d, v_d = scratch

    for h in range(NHL):
        k_sb = p2.tile([P, S], BF, tag="k_sb")
        nc.sync.dma_start(k_sb[:], kt_d[bass.ts(h, P), :])
        q_sb = p2.tile([P, S], BF, tag="q_sb")
        nc.sync.dma_start(q_sb[:], qt_d[bass.ts(h, P), :])
        v_sb = p2.tile([P, S // P, P], BF, tag="v_sb")
        nc.sync.dma_start(
            v_sb[:], v_d[:, bass.ts(h, P)].rearrange("(kt p) d -> p kt d", p=P))
        for qt in range(S // QT):
            blocks = spec[qt]
            nb = len(blocks)
            psum_sum = psSum.tile([P, QT], F32, tag="sum")
            psum_ctx = psC.tile([P, QT], F32, tag="ctx")
            for bi, (kt, qoff, mi) in enumerate(blocks):
                w = QT - qoff
                qsl = bass.ds(qt * QT + qoff, w)
                psum_s = psS.tile([P, QT], F32, tag="s")
                nc.tensor.matmul(
                    psum_s[:, 0:w], k_sb[:, bass.ts(kt, KT)],
                    q_sb[:, qsl], start=True, stop=True)
                if mi >= 0:
                    nc.vector.tensor_tensor(
                        psum_s[:, 0:w], psum_s[:, 0:w],
                        mask_r[:, mi, bass.ds(qoff, w)], mybir.AluOpType.add)
                e_sb = p2e.tile([P, QT], BF, tag="e")
                nc.scalar.activation(e_sb[:, 0:w], psum_s[:, 0:w], AF.Exp,
                                     scale=float(1.0 / np.sqrt(HD)))
                nc.tensor.matmul(psum_sum[:, bass.ds(qoff, w)], ones_r[:],
                                 e_sb[:, 0:w], start=(bi == 0),
                                 stop=(bi == nb - 1), skip_group_check=True)
                nc.tensor.matmul(psum_ctx[:, bass.ds(qoff, w)], v_sb[:, kt, :],
                                 e_sb[:, 0:w], start=(bi == 0),
                                 stop=(bi == nb - 1), skip_group_check=True)
            recip = p2e.tile([P, QT], F32, tag="recip")
            nc.vector.reciprocal(recip[:], psum_sum[:])
            nc.vector.tensor_mul(
                ctxT[:, h, bass.ts(qt, QT)], psum_ctx[:], recip[:])


def _phase3(nc, tc, pools, wo3, ctx_tiles, ot):
    """O^T partial = Wo-tile^T @ ctx^T. hs is the outer loop within each
    group of 4 (b, qt) columns so each Wo stationary tile is loaded into
    the PE array once and reused by 4 non-self-loading matmuls (the sim
    charges Ldweights as free, hardware does not)."""
    p3w, p3o, psO = pools
    bq = [(b, qt) for b in range(B) for qt in range(S // QT)]
    for oi in range(H // P):
        wo_sb = p3w.tile([P, NHL, P], BF, tag="wo")
        nc.sync.dma_start(wo_sb[:], wo3[:, :, bass.ts(oi, P)])
        for half in range(2):
            grp = bq[half * 4:(half + 1) * 4]
            psums = [psO.tile([P, QT], F32, tag=f"o{i}", name=f"o{i}")
                     for i in range(4)]
            for hs in range(NHL):
                nc.tensor.ldweights(wo_sb[:, hs, :])
                for i, (b, qt) in enumerate(grp):
                    mm = nc.tensor.matmul(
                        psums[i][:], wo_sb[:, hs, :],
                        ctx_tiles[b][:, hs, bass.ts(qt, QT)],
                        start=(hs == 0), stop=(hs == NHL - 1))
                    mm.ins.ldweights = False
            for i, (b, qt) in enumerate(grp):
                o_sb = p3o.tile([P, QT], BF, tag="o_sb")
                nc.scalar.copy(o_sb[:], psums[i][:])
                nc.sync.dma_start(
                    ot[bass.ts(oi, P), bass.ds(b * S + qt * QT, QT)], o_sb[:])


def _build(specs, n_mb, n_mask, reps=1, phases=(1, 2, 3)):
    nc = bacc.Bacc()

    xt = nc.declare_dram_parameter("xt", [H, BT], BF, isOutput=False)
    wqt = nc.declare_dram_parameter("wqt", [H, DL], BF, isOutput=False)
    wkt = nc.declare_dram_parameter("wkt", [H, DL], BF, isOutput=False)
    wvt = nc.declare_dram_parameter("wvt", [H, DL], BF, isOutput=False)
    wot = nc.declare_dram_parameter("wot", [DL, H], BF, isOutput=False)
    maskt = nc.declare_dram_parameter(
        "maskt", [max(n_mask, 1), KT, QT], BF, isOutput=False)
    cost = nc.declare_dram_parameter("cost", [S, DL], BF, isOutput=False)
    sint = nc.declare_dram_parameter("sint", [S, DL], BF, isOutput=False)
    identp = nc.declare_dram_parameter("identp", [P, P], BF, isOutput=False)
    ot = nc.declare_dram_parameter("ot", [H, BT], BF, isOutput=True)

    xt3 = xt.rearrange("(ho p) t -> p ho t", p=P)
    wq3 = wqt.rearrange("(ho p) d -> p ho d", p=P)
    wk3 = wkt.rearrange("(ho p) d -> p ho d", p=P)
    wv3 = wvt.rearrange("(ho p) d -> p ho d", p=P)
    wo3 = wot.rearrange("(hs p) o -> p hs o", p=P)
    mask3 = maskt.rearrange("n p q -> p n q")

    import contextlib

    with tile.TileContext(nc) as tc:
        with (
            tc.tile_pool(name="glob", bufs=1) as glob,
            tc.tile_pool(name="dram", bufs=1, space="DRAM") as dram,
        ):
            scratches = []
            for b in range(B):
                qd = dram.tile([DL, S], BF, tag=f"qt_d{b}", name=f"qt_d{b}")
                kd = dram.tile([DL, S], BF, tag=f"kt_d{b}", name=f"kt_d{b}")
                vd = dram.tile([S, DL], BF, tag=f"v_d{b}", name=f"v_d{b}")
                scratches.append((qd, kd, vd))

            ones_f = glob.tile([P, P], F32, tag="ones_f")
            nc.any.memset(ones_f[:], 1.0)
            ones_r = glob.tile([P, P], BF, tag="ones_r")
            nc.vector.tensor_copy(ones_r[:], ones_f[:])
            ident = glob.tile([P, P], BF, tag="ident")
            nc.sync.dma_start(ident[:], identp[:, :])

            loop_cm = tc.For_i(0, reps, 1) if reps > 1 else contextlib.nullcontext()
            with loop_cm, tc.tile_pool(name="p2m", bufs=1) as p2m:
                mask_r = p2m.tile([P, max(n_mask, 1), QT], BF,
                                  tag="mask_r", name="mask_r")

                def mask_load():
                    nc.sync.dma_start(mask_r[:], mask3[:, :, :])

                if 1 in phases:
                    with (
                        tc.tile_pool(name="p1x", bufs=2) as p1x,
                        tc.tile_pool(name="p1w", bufs=1) as p1w,
                        tc.tile_pool(name="p1t", bufs=2) as p1t,
                        tc.tile_pool(name="p1r", bufs=2) as p1r,
                        tc.tile_pool(name="psA", bufs=2, space="PSUM") as psA,
                        tc.tile_pool(name="psV", bufs=2, space="PSUM") as psV,
                        tc.tile_pool(name="psT", bufs=2, space="PSUM") as psT,
                    ):
                        tabs_ap = {"c": cost, "s": sint}
                        _phase1(nc, tc, (p1x, p1w, p1t, p1r, psA, psV, psT),
                                (xt3, wq3, wk3, wv3, tabs_ap, ident), scratches,
                                mask_load)
                if 2 in phases:
                    with tc.tile_pool(name="ctxp", bufs=1) as ctxp:
                        ctx_tiles = []
                        with (
                            tc.tile_pool(name="p2", bufs=2) as p2,
                            tc.tile_pool(name="p2e", bufs=3) as p2e,
                            tc.tile_pool(name="psS", bufs=3, space="PSUM") as psS,
                            tc.tile_pool(name="psSum", bufs=2, space="PSUM") as psSum,
                            tc.tile_pool(name="psC", bufs=2, space="PSUM") as psC,
                        ):
                            for b in range(B):
                                mb = b % n_mb
                                ctxT = ctxp.tile([P, NHL, S], BF, tag=f"ctxT{b}",
                                                 name=f"ctxT{b}")
                                ctx_tiles.append(ctxT)
                                _phase2_batch(
                                    nc, tc, b, specs[mb],
                                    (p2, p2e, psS, psSum, psC),
                                    mask_r, ones_r, scratches[b], ctxT)
                        if 3 in phases:
                            with (
                                tc.tile_pool(name="p3w", bufs=3) as p3w,
                                tc.tile_pool(name="p3o", bufs=4) as p3o,
                                tc.tile_pool(name="psO", bufs=2, space="PSUM") as psO,
                            ):
                                _phase3(nc, tc, (p3w, p3o, psO), wo3,
                                        ctx_tiles, ot)
    nc.finalize()
    return nc


def _rope_tables():
    inv_freq = 1.0 / (10000.0 ** (np.arange(0, HD, 2, dtype=np.float32) / HD))
    t = np.arange(S, dtype=np.float32)
    freqs = np.einsum("i,j->ij", t, inv_freq)
    emb = np.concatenate([freqs, freqs], axis=-1)        # [S, HD]
    return np.cos(emb).astype(np.float32), np.sin(emb).astype(np.float32)


def _block_spec(masks, n_mb):
    """masks: [n_mb, S, S] additive (q, k). Returns (specs, mask_tiles).

    specs[mb][qt] = list of (kt, qoff, mi): kt key tile, qoff first valid
    query column (rows below it are fully masked in this block), mi index
    into mask_tiles ([n, KT, QT], k-major) or -1 if the block needs no mask.
    """
    specs, tiles = [], []
    for mb in range(n_mb):
        mask = masks[mb]
        spec = []
        for qt in range(S // QT):
            row = []
            sub_q = mask[qt * QT:(qt + 1) * QT]          # [QT, S]
            for kt in range(S // KT):
                blk = sub_q[:, kt * KT:(kt + 1) * KT]    # [QT, KT]
                full = np.all(blk <= -1e8, axis=1)       # fully-masked q rows
                if full.all():
                    continue
                nz = np.flatnonzero(~full)
                qoff = int(nz[0])
                assert full[:qoff].all() and not full[qoff:].any(), \
                    "non-contiguous masked q rows not supported"
                if np.any(blk[qoff:] != 0.0):
                    tiles.append(np.ascontiguousarray(blk.T))  # [KT, QT]
                    mi = len(tiles) - 1
                else:
                    mi = -1
                row.append((kt, qoff, mi))
            assert row, "a query tile with all keys masked is not supported"
            spec.append(row)
        specs.append(spec)
    mask_tiles = (np.stack(tiles) if tiles
                  else np.zeros((1, KT, QT), np.float32))
    return specs, mask_tiles.astype(np.float32)


def _prepare(hidden_states, attention_mask, Wq, Wk, Wv, Wo):
    """Host-side marshaling. Returns (specs, n_mb, n_mask, in_maps)."""
    hidden_states = np.asarray(hidden_states, dtype=np.float32)
    attention_mask = np.asarray(attention_mask, dtype=np.float32)

    xt = np.ascontiguousarray(
        hidden_states.reshape(BT, H).T).astype(NPBF)            # [H, BT]
    wqT = np.ascontiguousarray(np.asarray(Wq, np.float32).T).astype(NPBF)
    wkT = np.ascontiguousarray(np.asarray(Wk, np.float32).T).astype(NPBF)
    wvT = np.ascontiguousarray(np.asarray(Wv, np.float32).T).astype(NPBF)
    woT = np.ascontiguousarray(np.asarray(Wo, np.float32).T).astype(NPBF)

    masks = attention_mask[:, 0]                                # [B, S, S]
    same = bool(np.array_equal(masks[0], masks[1])) if B == 2 else True
    n_mb = 1 if same else B
    specs, mask_tiles = _block_spec(masks, n_mb)
    n_mask = mask_tiles.shape[0]

    cos, sin = _rope_tables()                    # [S, HD]
    # rotate-half sign folded into sin (first half -sin, second +sin); the
    # 1/sqrt(HD) score scale is applied by the exp activation instead, so Q
    # and K share one table pair (mask tiles carry sqrt(HD)).
    sin_eff = sin.copy()
    sin_eff[:, :HD // 2] *= -1.0
    cost = np.ascontiguousarray(np.tile(cos, (1, NHL))).astype(NPBF)
    sint = np.ascontiguousarray(np.tile(sin_eff, (1, NHL))).astype(NPBF)

    in_maps = []
    for g in range(NC):
        dsl = slice(g * DL, (g + 1) * DL)
        in_maps.append({
            "xt": xt,
            "wqt": np.ascontiguousarray(wqT[:, dsl]),
            "wkt": np.ascontiguousarray(wkT[:, dsl]),
            "wvt": np.ascontiguousarray(wvT[:, dsl]),
            "wot": np.ascontiguousarray(woT[dsl, :]),
            "maskt": (mask_tiles * np.sqrt(np.float32(HD))).astype(NPBF),
            "cost": cost, "sint": sint,
            "identp": np.eye(P, dtype=np.float32).astype(NPBF),
        })
    return specs, n_mb, n_mask, in_maps


_CACHE = {}


def kernel(hidden_states, attention_mask, Wq, Wk, Wv, Wo):
    from concourse.bass_utils import run_bass_kernel_spmd

    specs, n_mb, n_mask, in_maps = _prepare(
        hidden_states, attention_mask, Wq, Wk, Wv, Wo)

    key = (n_mb, n_mask, tuple(tuple(map(tuple, s)) for s in specs))
    if key not in _CACHE:
        _CACHE[key] = _build(specs, n_mb, n_mask)
    nc = _CACHE[key]

    try:
        res = run_bass_kernel_spmd(nc, in_maps, list(range(NC)), trace=False)
    except Exception:
        # one retry: a wedged NeuronCore usually recovers on re-dispatch
        import time as _time
        _time.sleep(5)
        res = run_bass_kernel_spmd(nc, in_maps, list(range(NC)), trace=False)
    acc = np.zeros((H, BT), dtype=np.float32)
    for g in range(NC):
        acc += res.results[g]["ot"].astype(np.float32)
    return np.ascontiguousarray(acc.T).reshape(B, S, H)


# revision 26
# speedup vs baseline: 1.0679x; 1.0679x over previous
"""LlamaAttention (B=2, S=2048, H=4096, NH=32) on 8 Trainium2 NeuronCores.

Sharding: tensor-parallel over heads (4 heads / core). Column-parallel
Wq/Wk/Wv, row-parallel Wo; the Wo partial sums are reduced on the host
(the all-reduce of the TP recipe, done during unshard).

All bulk tensors move as bf16 (matmul is full-rate for bf16, so this
halves HBM traffic at no compute cost); accumulation stays fp32 in PSUM.

Per-core dataflow:
  phase 1: Wq/Wk/Wv resident in SBUF (bf16); X^T streamed once in
           [4096, 512]-token chunks. Q^T,K^T = RoPE(W^T-chunk @ X^T)
           -> DRAM [d, t]; V = X^T-chunk^T @ WvT -> DRAM [t, d].
  phase 2: per head: S^T[k,q] = K^T-tile^T @ Q^T (contraction d=128).
           Only non-fully-masked 128(k)x512(q) blocks are computed, and
           diagonal blocks are trimmed to their valid q-subrange. The 16
           distinct diagonal mask tiles are cached in SBUF. exp on ACT;
           denominators via ones-matmul; ctx^T[d,q] = V-tile^T @ expS^T.
  phase 3: O^T partial = WoT-tile^T @ ctx^T -> DRAM [o, t] bf16, with the
           Wo tile loop outermost so Wo is loaded exactly once.

Host side: pre-transposes X and the weights (layout marshaling, bf16
cast), builds the block structure from the attention mask, sums the 8
partial O^T outputs in fp32 and transposes back.
"""
import sys

sys.path.insert(0, "/opt/trn_rl_repo")

import numpy as np
import ml_dtypes

import concourse.bass as bass
import concourse.bacc as bacc
import concourse.tile as tile
import concourse.mybir as mybir

B, S, H, NH = 2, 2048, 4096, 32
HD = H // NH          # 128
NC = 8                # cores
DL = H // NC          # 512 local dims (4 heads / core)
NHL = NH // NC        # 4 local heads
BT = B * S            # 4096 tokens
P = 128
CH = 512              # phase-1 X^T chunk (matmul moving dim)
QT = 512              # phase-2 query tile (free dim)
KT = 128              # phase-2 key tile (partition dim)
NKO = H // P          # 32 contraction subtiles

DT = mybir.dt.float32
BF = mybir.dt.bfloat16
F32 = mybir.dt.float32
AF = mybir.ActivationFunctionType
NPBF = ml_dtypes.bfloat16


def _phase1(nc, tc, pools, aps, scratches, mask_load):
    """QKV projections + RoPE, streaming X once through resident weights."""
    p1x, p1w, p1t, p1r, psA, psV = pools
    xt3, wq3, wk3, wv3, tabs_ap = aps

    # resident weights (split into 4 tiles each so matmuls can start as
    # soon as the first quarter lands). DMA issue order is chosen so the
    # first Q matmuls (need wq + xc0) start ~25us in: wq, xc0, then
    # wk/tables/wv/xc1 land under the first chunk's Q compute.
    def _load_w(w3, nm):
        lst = []
        for q4 in range(4):
            t = p1w.tile([P, NKO // 4, DL], BF, tag=f"{nm}{q4}",
                         name=f"{nm}{q4}")
            nc.sync.dma_start(t[:], w3[:, bass.ds(q4 * (NKO // 4), NKO // 4), :])
            lst.append(t)
        return lst

    xcs = {}

    def _load_xc(c):
        xc = p1x.tile([P, NKO, CH], BF, tag="xt", name=f"xt{c}")
        nc.sync.dma_start(xc[:], xt3[:, :, bass.ds(c * CH, CH)])
        xcs[c] = xc

    wq_r = _load_w(wq3, "wq")
    _load_xc(0)
    wk_r = _load_w(wk3, "wk")
    # resident RoPE tables [HD, S] (cosq/sinq pre-scaled by 1/sqrt(HD))
    tabs = {}
    for nm, t_ap in tabs_ap.items():
        tt = p1t.tile([P, S], BF, tag="tab_" + nm, name="tab_" + nm)
        nc.sync.dma_start(tt[:], t_ap[:, :])
        tabs[nm] = tt
    wv_r = _load_w(wv3, "wv")
    mask_load()  # phase-2 mask tiles: queue behind phase-1-critical loads

    for c in range(BT // CH):                     # 8 chunks of 512 tokens
        b, tloc = c // (S // CH), (c % (S // CH)) * CH
        qt_d, kt_d, v_d = scratches[b]
        if c + 1 < BT // CH:
            _load_xc(c + 1)
        xc = xcs.pop(c)
        tsl = bass.ds(tloc, CH)
        # --- Q^T and K^T with RoPE ---
        for (w_r, cnm, snm, outd) in ((wq_r, "cq", "sq", qt_d),
                                      (wk_r, "ck", "sk", kt_d)):
            cosT, sinT = tabs[cnm], tabs[snm]
            for dsub in range(DL // P):           # 4 heads
                psum = psA.tile([P, CH], F32, tag="qk")
                for hs in range(NKO):
                    nc.tensor.matmul(
                        psum[:], w_r[hs // 8][:, hs % 8, bass.ts(dsub, P)],
                        xc[:, hs, :], start=(hs == 0), stop=(hs == NKO - 1))
                rc = p1r.tile([P, CH], BF, tag="rc")
                rs = p1r.tile([P, CH], F32, tag="rs")
                nc.vector.tensor_mul(rc[:], psum[:], cosT[:, tsl])
                nc.vector.tensor_mul(
                    rs[0:64, :], psum[64:128, :], sinT[0:64, tsl])
                nc.vector.tensor_mul(
                    rs[64:128, :], psum[0:64, :], sinT[64:128, tsl])
                nc.vector.tensor_tensor(
                    rc[0:64, :], rc[0:64, :], rs[0:64, :],
                    mybir.AluOpType.subtract)
                nc.vector.tensor_tensor(
                    rc[64:128, :], rc[64:128, :], rs[64:128, :],
                    mybir.AluOpType.add)
                nc.sync.dma_start(outd[bass.ts(dsub, P), tsl], rc[:])
        # --- V in [t, d] layout ---
        for j in range(CH // P):
            psum_v = psV.tile([P, DL], F32, tag="v")
            for hs in range(NKO):
                nc.tensor.matmul(
                    psum_v[:], xc[:, hs, bass.ts(j, P)],
                    wv_r[hs // 8][:, hs % 8, :],
                    start=(hs == 0), stop=(hs == NKO - 1))
            vo = p1r.tile([P, DL], BF, tag="vo")
            nc.scalar.copy(vo[:], psum_v[:])
            nc.sync.dma_start(v_d[bass.ds(tloc + j * P, P), :], vo[:])


def _phase2_batch(nc, tc, b, spec, pools, mask_r, ones_r, scratch, ctxT):
    """Attention for batch b -> ctxT [P, NHL, S] (bf16)."""
    p2, p2e, psS, psSum, psC = pools
    qt_d, kt_d, v_d = scratch

    for h in range(NHL):
        k_sb = p2.tile([P, S], BF, tag="k_sb")
        nc.sync.dma_start(k_sb[:], kt_d[bass.ts(h, P), :])
        q_sb = p2.tile([P, S], BF, tag="q_sb")
        nc.sync.dma_start(q_sb[:], qt_d[bass.ts(h, P), :])
        v_sb = p2.tile([P, S // P, P], BF, tag="v_sb")
        nc.sync.dma_start(
            v_sb[:], v_d[:, bass.ts(h, P)].rearrange("(kt p) d -> p kt d", p=P))
        for qt in range(S // QT):
            blocks = spec[qt]
            nb = len(blocks)
            psum_sum = psSum.tile([P, QT], F32, tag="sum")
            psum_ctx = psC.tile([P, QT], F32, tag="ctx")
            # exp outputs for the whole query tile live in one [P, nb, QT]
            # tile so the denominator matmuls can share a single load of the
            # constant ones stationary (hardware pays for each Ldweights).
            e_all = p2e.tile([P, S // KT, QT], BF, tag="e", name=f"e{qt}")
            for bi, (kt, qoff, mi) in enumerate(blocks):
                w = QT - qoff
                qsl = bass.ds(qt * QT + qoff, w)
                psum_s = psS.tile([P, QT], F32, tag="s")
                nc.tensor.matmul(
                    psum_s[:, 0:w], k_sb[:, bass.ts(kt, KT)],
                    q_sb[:, qsl], start=True, stop=True)
                if mi >= 0:
                    nc.vector.tensor_tensor(
                        psum_s[:, 0:w], psum_s[:, 0:w],
                        mask_r[:, mi, bass.ds(qoff, w)], mybir.AluOpType.add)
                nc.scalar.activation(e_all[:, bi, 0:w], psum_s[:, 0:w], AF.Exp)
            nc.tensor.ldweights(ones_r[:])
            for bi, (kt, qoff, mi) in enumerate(blocks):
                w = QT - qoff
                mm = nc.tensor.matmul(
                    psum_sum[:, bass.ds(qoff, w)], ones_r[:],
                    e_all[:, bi, 0:w], start=(bi == 0),
                    stop=(bi == nb - 1), skip_group_check=True)
                mm.ins.ldweights = False
            for bi, (kt, qoff, mi) in enumerate(blocks):
                w = QT - qoff
                nc.tensor.matmul(
                    psum_ctx[:, bass.ds(qoff, w)], v_sb[:, kt, :],
                    e_all[:, bi, 0:w], start=(bi == 0),
                    stop=(bi == nb - 1), skip_group_check=True)
            recip = p2e.tile([P, QT], F32, tag="recip")
            nc.vector.reciprocal(recip[:], psum_sum[:])
            nc.vector.tensor_mul(
                ctxT[:, h, bass.ts(qt, QT)], psum_ctx[:], recip[:])


def _phase3(nc, tc, pools, wo3, ctx_tiles, ot):
    """O^T partial = Wo-tile^T @ ctx^T. hs is the outer loop within each
    group of 4 (b, qt) columns so each Wo stationary tile is loaded into
    the PE array once and reused by 4 non-self-loading matmuls (the sim
    charges Ldweights as free, hardware does not)."""
    p3w, p3o, psO = pools
    bq = [(b, qt) for b in range(B) for qt in range(S // QT)]
    for oi in range(H // P):
        wo_sb = p3w.tile([P, NHL, P], BF, tag="wo")
        nc.sync.dma_start(wo_sb[:], wo3[:, :, bass.ts(oi, P)])
        for half in range(2):
            grp = bq[half * 4:(half + 1) * 4]
            psums = [psO.tile([P, QT], F32, tag=f"o{i}", name=f"o{i}")
                     for i in range(4)]
            for hs in range(NHL):
                nc.tensor.ldweights(wo_sb[:, hs, :])
                for i, (b, qt) in enumerate(grp):
                    mm = nc.tensor.matmul(
                        psums[i][:], wo_sb[:, hs, :],
                        ctx_tiles[b][:, hs, bass.ts(qt, QT)],
                        start=(hs == 0), stop=(hs == NHL - 1))
                    mm.ins.ldweights = False
            for i, (b, qt) in enumerate(grp):
                o_sb = p3o.tile([P, QT], BF, tag="o_sb")
                nc.scalar.copy(o_sb[:], psums[i][:])
                nc.sync.dma_start(
                    ot[bass.ts(oi, P), bass.ds(b * S + qt * QT, QT)], o_sb[:])


def _build(specs, n_mb, n_mask, reps=1, phases=(1, 2, 3)):
    nc = bacc.Bacc()

    xt = nc.declare_dram_parameter("xt", [H, BT], BF, isOutput=False)
    wqt = nc.declare_dram_parameter("wqt", [H, DL], BF, isOutput=False)
    wkt = nc.declare_dram_parameter("wkt", [H, DL], BF, isOutput=False)
    wvt = nc.declare_dram_parameter("wvt", [H, DL], BF, isOutput=False)
    wot = nc.declare_dram_parameter("wot", [DL, H], BF, isOutput=False)
    maskt = nc.declare_dram_parameter(
        "maskt", [max(n_mask, 1), KT, QT], BF, isOutput=False)
    cosq = nc.declare_dram_parameter("cosq", [HD, S], BF, isOutput=False)
    sinq = nc.declare_dram_parameter("sinq", [HD, S], BF, isOutput=False)
    cosk = nc.declare_dram_parameter("cosk", [HD, S], BF, isOutput=False)
    sink = nc.declare_dram_parameter("sink", [HD, S], BF, isOutput=False)
    ot = nc.declare_dram_parameter("ot", [H, BT], BF, isOutput=True)

    xt3 = xt.rearrange("(ho p) t -> p ho t", p=P)
    wq3 = wqt.rearrange("(ho p) d -> p ho d", p=P)
    wk3 = wkt.rearrange("(ho p) d -> p ho d", p=P)
    wv3 = wvt.rearrange("(ho p) d -> p ho d", p=P)
    wo3 = wot.rearrange("(hs p) o -> p hs o", p=P)
    mask3 = maskt.rearrange("n p q -> p n q")

    import contextlib

    with tile.TileContext(nc) as tc:
        with (
            tc.tile_pool(name="glob", bufs=1) as glob,
            tc.tile_pool(name="dram", bufs=1, space="DRAM") as dram,
        ):
            scratches = []
            for b in range(B):
                qd = dram.tile([DL, S], BF, tag=f"qt_d{b}", name=f"qt_d{b}")
                kd = dram.tile([DL, S], BF, tag=f"kt_d{b}", name=f"kt_d{b}")
                vd = dram.tile([S, DL], BF, tag=f"v_d{b}", name=f"v_d{b}")
                scratches.append((qd, kd, vd))

            ones_f = glob.tile([P, P], F32, tag="ones_f")
            nc.any.memset(ones_f[:], 1.0)
            ones_r = glob.tile([P, P], BF, tag="ones_r")
            nc.vector.tensor_copy(ones_r[:], ones_f[:])

            loop_cm = tc.For_i(0, reps, 1) if reps > 1 else contextlib.nullcontext()
            with loop_cm, tc.tile_pool(name="p2m", bufs=1) as p2m:
                mask_r = p2m.tile([P, max(n_mask, 1), QT], BF,
                                  tag="mask_r", name="mask_r")

                def mask_load():
                    nc.sync.dma_start(mask_r[:], mask3[:, :, :])

                if 1 in phases:
                    with (
                        tc.tile_pool(name="p1x", bufs=2) as p1x,
                        tc.tile_pool(name="p1w", bufs=1) as p1w,
                        tc.tile_pool(name="p1t", bufs=1) as p1t,
                        tc.tile_pool(name="p1r", bufs=3) as p1r,
                        tc.tile_pool(name="psA", bufs=3, space="PSUM") as psA,
                        tc.tile_pool(name="psV", bufs=3, space="PSUM") as psV,
                    ):
                        tabs_ap = {"cq": cosq, "sq": sinq, "ck": cosk, "sk": sink}
                        _phase1(nc, tc, (p1x, p1w, p1t, p1r, psA, psV),
                                (xt3, wq3, wk3, wv3, tabs_ap), scratches,
                                mask_load)
                if 2 in phases:
                    with tc.tile_pool(name="ctxp", bufs=1) as ctxp:
                        ctx_tiles = []
                        with (
                            tc.tile_pool(name="p2", bufs=2) as p2,
                            tc.tile_pool(name="p2e", bufs=2) as p2e,
                            tc.tile_pool(name="psS", bufs=4, space="PSUM") as psS,
                            tc.tile_pool(name="psSum", bufs=2, space="PSUM") as psSum,
                            tc.tile_pool(name="psC", bufs=2, space="PSUM") as psC,
                        ):
                            for b in range(B):
                                mb = b % n_mb
                                ctxT = ctxp.tile([P, NHL, S], BF, tag=f"ctxT{b}",
                                                 name=f"ctxT{b}")
                                ctx_tiles.append(ctxT)
                                _phase2_batch(
                                    nc, tc, b, specs[mb],
                                    (p2, p2e, psS, psSum, psC),
                                    mask_r, ones_r, scratches[b], ctxT)
                        if 3 in phases:
                            with (
                                tc.tile_pool(name="p3w", bufs=3) as p3w,
                                tc.tile_pool(name="p3o", bufs=4) as p3o,
                                tc.tile_pool(name="psO", bufs=2, space="PSUM") as psO,
                            ):
                                _phase3(nc, tc, (p3w, p3o, psO), wo3,
                                        ctx_tiles, ot)
    nc.finalize()
    return nc


def _rope_tables():
    inv_freq = 1.0 / (10000.0 ** (np.arange(0, HD, 2, dtype=np.float32) / HD))
    t = np.arange(S, dtype=np.float32)
    freqs = np.einsum("i,j->ij", t, inv_freq)
    emb = np.concatenate([freqs, freqs], axis=-1)        # [S, HD]
    return np.cos(emb).astype(np.float32), np.sin(emb).astype(np.float32)


def _block_spec(masks, n_mb):
    """masks: [n_mb, S, S] additive (q, k). Returns (specs, mask_tiles).

    specs[mb][qt] = list of (kt, qoff, mi): kt key tile, qoff first valid
    query column (rows below it are fully masked in this block), mi index
    into mask_tiles ([n, KT, QT], k-major) or -1 if the block needs no mask.
    """
    specs, tiles = [], []
    for mb in range(n_mb):
        mask = masks[mb]
        spec = []
        for qt in range(S // QT):
            row = []
            sub_q = mask[qt * QT:(qt + 1) * QT]          # [QT, S]
            for kt in range(S // KT):
                blk = sub_q[:, kt * KT:(kt + 1) * KT]    # [QT, KT]
                full = np.all(blk <= -1e8, axis=1)       # fully-masked q rows
                if full.all():
                    continue
                nz = np.flatnonzero(~full)
                qoff = int(nz[0])
                assert full[:qoff].all() and not full[qoff:].any(), \
                    "non-contiguous masked q rows not supported"
                if np.any(blk[qoff:] != 0.0):
                    tiles.append(np.ascontiguousarray(blk.T))  # [KT, QT]
                    mi = len(tiles) - 1
                else:
                    mi = -1
                row.append((kt, qoff, mi))
            assert row, "a query tile with all keys masked is not supported"
            spec.append(row)
        specs.append(spec)
    mask_tiles = (np.stack(tiles) if tiles
                  else np.zeros((1, KT, QT), np.float32))
    return specs, mask_tiles.astype(np.float32)


def _prepare(hidden_states, attention_mask, Wq, Wk, Wv, Wo):
    """Host-side marshaling. Returns (specs, n_mb, n_mask, in_maps)."""
    hidden_states = np.asarray(hidden_states, dtype=np.float32)
    attention_mask = np.asarray(attention_mask, dtype=np.float32)

    xt = np.ascontiguousarray(
        hidden_states.reshape(BT, H).T).astype(NPBF)            # [H, BT]
    wqT = np.ascontiguousarray(np.asarray(Wq, np.float32).T).astype(NPBF)
    wkT = np.ascontiguousarray(np.asarray(Wk, np.float32).T).astype(NPBF)
    wvT = np.ascontiguousarray(np.asarray(Wv, np.float32).T).astype(NPBF)
    woT = np.ascontiguousarray(np.asarray(Wo, np.float32).T).astype(NPBF)

    masks = attention_mask[:, 0]                                # [B, S, S]
    same = bool(np.array_equal(masks[0], masks[1])) if B == 2 else True
    n_mb = 1 if same else B
    specs, mask_tiles = _block_spec(masks, n_mb)
    n_mask = mask_tiles.shape[0]

    cos, sin = _rope_tables()
    scale = 1.0 / np.sqrt(np.float32(HD))
    cosq = np.ascontiguousarray((cos * scale).T).astype(NPBF)   # [HD, S]
    sinq = np.ascontiguousarray((sin * scale).T).astype(NPBF)
    cosk = np.ascontiguousarray(cos.T).astype(NPBF)
    sink = np.ascontiguousarray(sin.T).astype(NPBF)

    in_maps = []
    for g in range(NC):
        dsl = slice(g * DL, (g + 1) * DL)
        in_maps.append({
            "xt": xt,
            "wqt": np.ascontiguousarray(wqT[:, dsl]),
            "wkt": np.ascontiguousarray(wkT[:, dsl]),
            "wvt": np.ascontiguousarray(wvT[:, dsl]),
            "wot": np.ascontiguousarray(woT[dsl, :]),
            "maskt": mask_tiles.astype(NPBF),
            "cosq": cosq, "sinq": sinq, "cosk": cosk, "sink": sink,
        })
    return specs, n_mb, n_mask, in_maps


_CACHE = {}


def kernel(hidden_states, attention_mask, Wq, Wk, Wv, Wo):
    from concourse.bass_utils import run_bass_kernel_spmd

    specs, n_mb, n_mask, in_maps = _prepare(
        hidden_states, attention_mask, Wq, Wk, Wv, Wo)

    key = (n_mb, n_mask, tuple(tuple(map(tuple, s)) for s in specs))
    if key not in _CACHE:
        _CACHE[key] = _build(specs, n_mb, n_mask)
    nc = _CACHE[key]

    try:
        res = run_bass_kernel_spmd(nc, in_maps, list(range(NC)), trace=False)
    except Exception:
        # one retry: a wedged NeuronCore usually recovers on re-dispatch
        import time as _time
        _time.sleep(5)
        res = run_bass_kernel_spmd(nc, in_maps, list(range(NC)), trace=False)
    acc = np.zeros((H, BT), dtype=np.float32)
    for g in range(NC):
        acc += res.results[g]["ot"].astype(np.float32)
    return np.ascontiguousarray(acc.T).reshape(B, S, H)


# revision 27
# speedup vs baseline: 1.0741x; 1.0058x over previous
"""LlamaAttention (B=2, S=2048, H=4096, NH=32) on 8 Trainium2 NeuronCores.

Sharding: tensor-parallel over heads (4 heads / core). Column-parallel
Wq/Wk/Wv, row-parallel Wo; the Wo partial sums are reduced on the host
(the all-reduce of the TP recipe, done during unshard).

All bulk tensors move as bf16 (matmul is full-rate for bf16, so this
halves HBM traffic at no compute cost); accumulation stays fp32 in PSUM.

Per-core dataflow:
  phase 1: Wq/Wk/Wv resident in SBUF (bf16); X^T streamed once in
           [4096, 512]-token chunks. Q^T,K^T = RoPE(W^T-chunk @ X^T)
           -> DRAM [d, t]; V = X^T-chunk^T @ WvT -> DRAM [t, d].
  phase 2: per head: S^T[k,q] = K^T-tile^T @ Q^T (contraction d=128).
           Only non-fully-masked 128(k)x512(q) blocks are computed, and
           diagonal blocks are trimmed to their valid q-subrange. The 16
           distinct diagonal mask tiles are cached in SBUF. exp on ACT;
           denominators via ones-matmul; ctx^T[d,q] = V-tile^T @ expS^T.
  phase 3: O^T partial = WoT-tile^T @ ctx^T -> DRAM [o, t] bf16, with the
           Wo tile loop outermost so Wo is loaded exactly once.

Host side: pre-transposes X and the weights (layout marshaling, bf16
cast), builds the block structure from the attention mask, sums the 8
partial O^T outputs in fp32 and transposes back.
"""
import sys

sys.path.insert(0, "/opt/trn_rl_repo")

import numpy as np
import ml_dtypes

import concourse.bass as bass
import concourse.bacc as bacc
import concourse.tile as tile
import concourse.mybir as mybir

B, S, H, NH = 2, 2048, 4096, 32
HD = H // NH          # 128
NC = 8                # cores
DL = H // NC          # 512 local dims (4 heads / core)
NHL = NH // NC        # 4 local heads
BT = B * S            # 4096 tokens
P = 128
CH = 512              # phase-1 X^T chunk (matmul moving dim)
QT = 512              # phase-2 query tile (free dim)
KT = 128              # phase-2 key tile (partition dim)
NKO = H // P          # 32 contraction subtiles

DT = mybir.dt.float32
BF = mybir.dt.bfloat16
F32 = mybir.dt.float32
AF = mybir.ActivationFunctionType
NPBF = ml_dtypes.bfloat16


def _phase1(nc, tc, pools, aps, scratches, mask_load):
    """QKV projections + RoPE, streaming X once through resident weights."""
    p1x, p1w, p1t, p1r, psA, psV = pools
    xt3, wq3, wk3, wv3, tabs_ap = aps

    # resident weights (split into 4 tiles each so matmuls can start as
    # soon as the first quarter lands). DMA issue order is chosen so the
    # first Q matmuls (need wq + xc0) start ~25us in: wq, xc0, then
    # wk/tables/wv/xc1 land under the first chunk's Q compute.
    def _load_w(w3, nm):
        lst = []
        for q4 in range(4):
            t = p1w.tile([P, NKO // 4, DL], BF, tag=f"{nm}{q4}",
                         name=f"{nm}{q4}")
            nc.sync.dma_start(t[:], w3[:, bass.ds(q4 * (NKO // 4), NKO // 4), :])
            lst.append(t)
        return lst

    xcs = {}

    def _load_xc(c):
        xc = p1x.tile([P, NKO, CH], BF, tag="xt", name=f"xt{c}")
        nc.sync.dma_start(xc[:], xt3[:, :, bass.ds(c * CH, CH)])
        xcs[c] = xc

    wq_r = _load_w(wq3, "wq")
    _load_xc(0)
    wk_r = _load_w(wk3, "wk")
    # resident RoPE tables [HD, S] (cosq/sinq pre-scaled by 1/sqrt(HD))
    tabs = {}
    for nm, t_ap in tabs_ap.items():
        tt = p1t.tile([P, S], BF, tag="tab_" + nm, name="tab_" + nm)
        nc.sync.dma_start(tt[:], t_ap[:, :])
        tabs[nm] = tt
    wv_r = _load_w(wv3, "wv")
    mask_load()  # phase-2 mask tiles: queue behind phase-1-critical loads

    for c in range(BT // CH):                     # 8 chunks of 512 tokens
        b, tloc = c // (S // CH), (c % (S // CH)) * CH
        qt_d, kt_d, v_d = scratches[b]
        if c + 1 < BT // CH:
            _load_xc(c + 1)
        xc = xcs.pop(c)
        tsl = bass.ds(tloc, CH)
        # --- Q^T and K^T with RoPE ---
        for (w_r, cnm, snm, outd) in ((wq_r, "cq", "sq", qt_d),
                                      (wk_r, "ck", "sk", kt_d)):
            cosT, sinT = tabs[cnm], tabs[snm]
            for dsub in range(DL // P):           # 4 heads
                psum = psA.tile([P, CH], F32, tag="qk")
                for hs in range(NKO):
                    nc.tensor.matmul(
                        psum[:], w_r[hs // 8][:, hs % 8, bass.ts(dsub, P)],
                        xc[:, hs, :], start=(hs == 0), stop=(hs == NKO - 1))
                rc = p1r.tile([P, CH], BF, tag="rc")
                rs = p1r.tile([P, CH], F32, tag="rs")
                nc.vector.tensor_mul(rc[:], psum[:], cosT[:, tsl])
                nc.vector.tensor_mul(
                    rs[0:64, :], psum[64:128, :], sinT[0:64, tsl])
                nc.vector.tensor_mul(
                    rs[64:128, :], psum[0:64, :], sinT[64:128, tsl])
                nc.vector.tensor_tensor(
                    rc[0:64, :], rc[0:64, :], rs[0:64, :],
                    mybir.AluOpType.subtract)
                nc.vector.tensor_tensor(
                    rc[64:128, :], rc[64:128, :], rs[64:128, :],
                    mybir.AluOpType.add)
                nc.sync.dma_start(outd[bass.ts(dsub, P), tsl], rc[:])
        # --- V in [t, d] layout ---
        for j in range(CH // P):
            psum_v = psV.tile([P, DL], F32, tag="v")
            for hs in range(NKO):
                nc.tensor.matmul(
                    psum_v[:], xc[:, hs, bass.ts(j, P)],
                    wv_r[hs // 8][:, hs % 8, :],
                    start=(hs == 0), stop=(hs == NKO - 1))
            vo = p1r.tile([P, DL], BF, tag="vo")
            nc.scalar.copy(vo[:], psum_v[:])
            nc.sync.dma_start(v_d[bass.ds(tloc + j * P, P), :], vo[:])


def _phase2_batch(nc, tc, b, spec, pools, mask_r, ones_r, scratch, ctxT):
    """Attention for batch b -> ctxT [P, NHL, S] (bf16)."""
    p2, p2e, psS, psSum, psC = pools
    qt_d, kt_d, v_d = scratch

    for h in range(NHL):
        k_sb = p2.tile([P, S], BF, tag="k_sb")
        nc.sync.dma_start(k_sb[:], kt_d[bass.ts(h, P), :])
        q_sb = p2.tile([P, S], BF, tag="q_sb")
        nc.sync.dma_start(q_sb[:], qt_d[bass.ts(h, P), :])
        v_sb = p2.tile([P, S // P, P], BF, tag="v_sb")
        nc.sync.dma_start(
            v_sb[:], v_d[:, bass.ts(h, P)].rearrange("(kt p) d -> p kt d", p=P))
        for qt in range(S // QT):
            blocks = spec[qt]
            nb = len(blocks)
            psum_sum = psSum.tile([P, QT], F32, tag="sum")
            psum_ctx = psC.tile([P, QT], F32, tag="ctx")
            # exp outputs for the whole query tile live in one [P, nb, QT]
            # tile so the denominator matmuls can share a single load of the
            # constant ones stationary (hardware pays for each Ldweights).
            e_all = p2e.tile([P, S // KT, QT], BF, tag="e", name=f"e{qt}")
            for bi, (kt, qoff, mi) in enumerate(blocks):
                w = QT - qoff
                qsl = bass.ds(qt * QT + qoff, w)
                psum_s = psS.tile([P, QT], F32, tag="s")
                nc.tensor.matmul(
                    psum_s[:, 0:w], k_sb[:, bass.ts(kt, KT)],
                    q_sb[:, qsl], start=True, stop=True)
                if mi >= 0:
                    nc.vector.tensor_tensor(
                        psum_s[:, 0:w], psum_s[:, 0:w],
                        mask_r[:, mi, bass.ds(qoff, w)], mybir.AluOpType.add)
                nc.scalar.activation(e_all[:, bi, 0:w], psum_s[:, 0:w], AF.Exp)
            nc.tensor.ldweights(ones_r[:])
            for bi, (kt, qoff, mi) in enumerate(blocks):
                w = QT - qoff
                mm = nc.tensor.matmul(
                    psum_sum[:, bass.ds(qoff, w)], ones_r[:],
                    e_all[:, bi, 0:w], start=(bi == 0),
                    stop=(bi == nb - 1), skip_group_check=True)
                mm.ins.ldweights = False
            for bi, (kt, qoff, mi) in enumerate(blocks):
                w = QT - qoff
                nc.tensor.matmul(
                    psum_ctx[:, bass.ds(qoff, w)], v_sb[:, kt, :],
                    e_all[:, bi, 0:w], start=(bi == 0),
                    stop=(bi == nb - 1), skip_group_check=True)
            recip = p2e.tile([P, QT], F32, tag="recip")
            nc.vector.reciprocal(recip[:], psum_sum[:])
            nc.vector.tensor_mul(
                ctxT[:, h, bass.ts(qt, QT)], psum_ctx[:], recip[:])


def _phase3(nc, tc, pools, wo3, ctx_tiles, ot):
    """O^T partial = Wo-tile^T @ ctx^T. hs is the outer loop within each
    group of 4 (b, qt) columns so each Wo stationary tile is loaded into
    the PE array once and reused by 4 non-self-loading matmuls (the sim
    charges Ldweights as free, hardware does not)."""
    p3w, p3o, psO = pools
    bq = [(b, qt) for b in range(B) for qt in range(S // QT)]
    for oi in range(H // P):
        wo_sb = p3w.tile([P, NHL, P], BF, tag="wo")
        nc.sync.dma_start(wo_sb[:], wo3[:, :, bass.ts(oi, P)])
        for half in range(2):
            grp = bq[half * 4:(half + 1) * 4]
            psums = [psO.tile([P, QT], F32, tag=f"o{i}", name=f"o{i}")
                     for i in range(4)]
            for hs in range(NHL):
                nc.tensor.ldweights(wo_sb[:, hs, :])
                for i, (b, qt) in enumerate(grp):
                    mm = nc.tensor.matmul(
                        psums[i][:], wo_sb[:, hs, :],
                        ctx_tiles[b][:, hs, bass.ts(qt, QT)],
                        start=(hs == 0), stop=(hs == NHL - 1))
                    mm.ins.ldweights = False
            for i, (b, qt) in enumerate(grp):
                o_sb = p3o.tile([P, QT], BF, tag="o_sb")
                nc.scalar.copy(o_sb[:], psums[i][:])
                nc.sync.dma_start(
                    ot[bass.ts(oi, P), bass.ds(b * S + qt * QT, QT)], o_sb[:])


def _build(specs, n_mb, n_mask, reps=1, phases=(1, 2, 3)):
    nc = bacc.Bacc()

    xt = nc.declare_dram_parameter("xt", [H, BT], BF, isOutput=False)
    wqt = nc.declare_dram_parameter("wqt", [H, DL], BF, isOutput=False)
    wkt = nc.declare_dram_parameter("wkt", [H, DL], BF, isOutput=False)
    wvt = nc.declare_dram_parameter("wvt", [H, DL], BF, isOutput=False)
    wot = nc.declare_dram_parameter("wot", [DL, H], BF, isOutput=False)
    maskt = nc.declare_dram_parameter(
        "maskt", [max(n_mask, 1), KT, QT], BF, isOutput=False)
    cosq = nc.declare_dram_parameter("cosq", [HD, S], BF, isOutput=False)
    sinq = nc.declare_dram_parameter("sinq", [HD, S], BF, isOutput=False)
    cosk = nc.declare_dram_parameter("cosk", [HD, S], BF, isOutput=False)
    sink = nc.declare_dram_parameter("sink", [HD, S], BF, isOutput=False)
    ot = nc.declare_dram_parameter("ot", [H, BT], BF, isOutput=True)

    xt3 = xt.rearrange("(ho p) t -> p ho t", p=P)
    wq3 = wqt.rearrange("(ho p) d -> p ho d", p=P)
    wk3 = wkt.rearrange("(ho p) d -> p ho d", p=P)
    wv3 = wvt.rearrange("(ho p) d -> p ho d", p=P)
    wo3 = wot.rearrange("(hs p) o -> p hs o", p=P)
    mask3 = maskt.rearrange("n p q -> p n q")

    import contextlib

    with tile.TileContext(nc) as tc:
        with (
            tc.tile_pool(name="glob", bufs=1) as glob,
            tc.tile_pool(name="dram", bufs=1, space="DRAM") as dram,
        ):
            scratches = []
            for b in range(B):
                qd = dram.tile([DL, S], BF, tag=f"qt_d{b}", name=f"qt_d{b}")
                kd = dram.tile([DL, S], BF, tag=f"kt_d{b}", name=f"kt_d{b}")
                vd = dram.tile([S, DL], BF, tag=f"v_d{b}", name=f"v_d{b}")
                scratches.append((qd, kd, vd))

            ones_f = glob.tile([P, P], F32, tag="ones_f")
            nc.any.memset(ones_f[:], 1.0)
            ones_r = glob.tile([P, P], BF, tag="ones_r")
            nc.vector.tensor_copy(ones_r[:], ones_f[:])

            loop_cm = tc.For_i(0, reps, 1) if reps > 1 else contextlib.nullcontext()
            with loop_cm, tc.tile_pool(name="p2m", bufs=1) as p2m:
                mask_r = p2m.tile([P, max(n_mask, 1), QT], BF,
                                  tag="mask_r", name="mask_r")

                def mask_load():
                    nc.sync.dma_start(mask_r[:], mask3[:, :, :])

                if 1 in phases:
                    with (
                        tc.tile_pool(name="p1x", bufs=2) as p1x,
                        tc.tile_pool(name="p1w", bufs=1) as p1w,
                        tc.tile_pool(name="p1t", bufs=1) as p1t,
                        tc.tile_pool(name="p1r", bufs=3) as p1r,
                        tc.tile_pool(name="psA", bufs=4, space="PSUM") as psA,
                        tc.tile_pool(name="psV", bufs=4, space="PSUM") as psV,
                    ):
                        tabs_ap = {"cq": cosq, "sq": sinq, "ck": cosk, "sk": sink}
                        _phase1(nc, tc, (p1x, p1w, p1t, p1r, psA, psV),
                                (xt3, wq3, wk3, wv3, tabs_ap), scratches,
                                mask_load)
                if 2 in phases:
                    with tc.tile_pool(name="ctxp", bufs=1) as ctxp:
                        ctx_tiles = []
                        with (
                            tc.tile_pool(name="p2", bufs=3) as p2,
                            tc.tile_pool(name="p2e", bufs=2) as p2e,
                            tc.tile_pool(name="psS", bufs=4, space="PSUM") as psS,
                            tc.tile_pool(name="psSum", bufs=2, space="PSUM") as psSum,
                            tc.tile_pool(name="psC", bufs=2, space="PSUM") as psC,
                        ):
                            for b in range(B):
                                mb = b % n_mb
                                ctxT = ctxp.tile([P, NHL, S], BF, tag=f"ctxT{b}",
                                                 name=f"ctxT{b}")
                                ctx_tiles.append(ctxT)
                                _phase2_batch(
                                    nc, tc, b, specs[mb],
                                    (p2, p2e, psS, psSum, psC),
                                    mask_r, ones_r, scratches[b], ctxT)
                        if 3 in phases:
                            with (
                                tc.tile_pool(name="p3w", bufs=4) as p3w,
                                tc.tile_pool(name="p3o", bufs=6) as p3o,
                                tc.tile_pool(name="psO", bufs=2, space="PSUM") as psO,
                            ):
                                _phase3(nc, tc, (p3w, p3o, psO), wo3,
                                        ctx_tiles, ot)
    nc.finalize()
    return nc


def _rope_tables():
    inv_freq = 1.0 / (10000.0 ** (np.arange(0, HD, 2, dtype=np.float32) / HD))
    t = np.arange(S, dtype=np.float32)
    freqs = np.einsum("i,j->ij", t, inv_freq)
    emb = np.concatenate([freqs, freqs], axis=-1)        # [S, HD]
    return np.cos(emb).astype(np.float32), np.sin(emb).astype(np.float32)


def _block_spec(masks, n_mb):
    """masks: [n_mb, S, S] additive (q, k). Returns (specs, mask_tiles).

    specs[mb][qt] = list of (kt, qoff, mi): kt key tile, qoff first valid
    query column (rows below it are fully masked in this block), mi index
    into mask_tiles ([n, KT, QT], k-major) or -1 if the block needs no mask.
    """
    specs, tiles = [], []
    for mb in range(n_mb):
        mask = masks[mb]
        spec = []
        for qt in range(S // QT):
            row = []
            sub_q = mask[qt * QT:(qt + 1) * QT]          # [QT, S]
            for kt in range(S // KT):
                blk = sub_q[:, kt * KT:(kt + 1) * KT]    # [QT, KT]
                full = np.all(blk <= -1e8, axis=1)       # fully-masked q rows
                if full.all():
                    continue
                nz = np.flatnonzero(~full)
                qoff = int(nz[0])
                assert full[:qoff].all() and not full[qoff:].any(), \
                    "non-contiguous masked q rows not supported"
                if np.any(blk[qoff:] != 0.0):
                    tiles.append(np.ascontiguousarray(blk.T))  # [KT, QT]
                    mi = len(tiles) - 1
                else:
                    mi = -1
                row.append((kt, qoff, mi))
            assert row, "a query tile with all keys masked is not supported"
            spec.append(row)
        specs.append(spec)
    mask_tiles = (np.stack(tiles) if tiles
                  else np.zeros((1, KT, QT), np.float32))
    return specs, mask_tiles.astype(np.float32)


def _prepare(hidden_states, attention_mask, Wq, Wk, Wv, Wo):
    """Host-side marshaling. Returns (specs, n_mb, n_mask, in_maps)."""
    hidden_states = np.asarray(hidden_states, dtype=np.float32)
    attention_mask = np.asarray(attention_mask, dtype=np.float32)

    xt = np.ascontiguousarray(
        hidden_states.reshape(BT, H).T).astype(NPBF)            # [H, BT]
    wqT = np.ascontiguousarray(np.asarray(Wq, np.float32).T).astype(NPBF)
    wkT = np.ascontiguousarray(np.asarray(Wk, np.float32).T).astype(NPBF)
    wvT = np.ascontiguousarray(np.asarray(Wv, np.float32).T).astype(NPBF)
    woT = np.ascontiguousarray(np.asarray(Wo, np.float32).T).astype(NPBF)

    masks = attention_mask[:, 0]                                # [B, S, S]
    same = bool(np.array_equal(masks[0], masks[1])) if B == 2 else True
    n_mb = 1 if same else B
    specs, mask_tiles = _block_spec(masks, n_mb)
    n_mask = mask_tiles.shape[0]

    cos, sin = _rope_tables()
    scale = 1.0 / np.sqrt(np.float32(HD))
    cosq = np.ascontiguousarray((cos * scale).T).astype(NPBF)   # [HD, S]
    sinq = np.ascontiguousarray((sin * scale).T).astype(NPBF)
    cosk = np.ascontiguousarray(cos.T).astype(NPBF)
    sink = np.ascontiguousarray(sin.T).astype(NPBF)

    in_maps = []
    for g in range(NC):
        dsl = slice(g * DL, (g + 1) * DL)
        in_maps.append({
            "xt": xt,
            "wqt": np.ascontiguousarray(wqT[:, dsl]),
            "wkt": np.ascontiguousarray(wkT[:, dsl]),
            "wvt": np.ascontiguousarray(wvT[:, dsl]),
            "wot": np.ascontiguousarray(woT[dsl, :]),
            "maskt": mask_tiles.astype(NPBF),
            "cosq": cosq, "sinq": sinq, "cosk": cosk, "sink": sink,
        })
    return specs, n_mb, n_mask, in_maps


_CACHE = {}


def kernel(hidden_states, attention_mask, Wq, Wk, Wv, Wo):
    from concourse.bass_utils import run_bass_kernel_spmd

    specs, n_mb, n_mask, in_maps = _prepare(
        hidden_states, attention_mask, Wq, Wk, Wv, Wo)

    key = (n_mb, n_mask, tuple(tuple(map(tuple, s)) for s in specs))
    if key not in _CACHE:
        _CACHE[key] = _build(specs, n_mb, n_mask)
    nc = _CACHE[key]

    try:
        res = run_bass_kernel_spmd(nc, in_maps, list(range(NC)), trace=False)
    except Exception:
        # one retry: a wedged NeuronCore usually recovers on re-dispatch
        import time as _time
        _time.sleep(5)
        res = run_bass_kernel_spmd(nc, in_maps, list(range(NC)), trace=False)
    acc = np.zeros((H, BT), dtype=np.float32)
    for g in range(NC):
        acc += res.results[g]["ot"].astype(np.float32)
    return np.ascontiguousarray(acc.T).reshape(B, S, H)
